# revision 25
# baseline (speedup 1.0000x reference)
"""Trainium2 Bass kernel for a quantized ResNet BasicBlock:

    out = relu(bn2(qconv2(relu(bn1(qconv1(x))))) + x)

where qconv = 3x3 conv (stride 1, pad 1) on 8-bit symmetric per-tensor
quantized activations/weights (wage-style, forward pass only), and bn is
training-mode BatchNorm2d (batch statistics over N,H,W).

Strategy (8 NeuronCores, data-parallel over batch):
  * Each core gets B/8 samples. Weights/BN params replicated.
  * Quantized values round(v/s*127) are integers in [-127,127] - exact in
    bfloat16 - so each 3x3 conv runs as 9 accumulated bf16 128x128 matmuls
    per output chunk (channels on the partition dim, shifted windows over a
    zero-padded spatial free dim with ONE shared pad column between rows),
    accumulating exactly in f32 PSUM. The (s_in*s_w/127^2) scale is folded
    into the BN affine transform.
  * All cross-core reductions are AllGather (single network phase) +
    local PE-transpose + reduce; a dummy warm-up collective issued at t=0
    overlaps the collectives-runtime init barrier with the x load.
  * round-to-nearest-even via the f32 magic-number trick (+1.5*2^23 then
    subtract), matching jnp.round.
"""

import numpy as np

import concourse.bass as bass
import concourse.bacc as bacc
import concourse.mybir as mybir
import concourse.tile as tile
from concourse import bass_isa
from concourse import bass_utils
from concourse.bass_interp import get_hw_module

f32 = mybir.dt.float32
bf16 = mybir.dt.bfloat16
AF = mybir.ActivationFunctionType
OP = mybir.AluOpType
AX = mybir.AxisListType

N_CORES = 8
MAGIC = 12582912.0  # 1.5 * 2^23: (t + MAGIC) - MAGIC == rint(t) for |t| < 2^22
EPS = 1e-5
QMAX = 127.0


def build_module(B=32, C=128, H=56, W=56, n_cores=N_CORES, rows_per_chunk=8):
    npc = B // n_cores          # samples per core
    HWl = H * W
    WP = W + 1                  # row pitch: W valid cols + ONE shared pad col
    XKLEN = (H + 2) * WP + 2    # padded image + head/tail guard elements
    RPC = rows_per_chunk
    assert H % RPC == 0
    NCH = H // RPC              # chunks (row groups) per sample
    CF = RPC * WP               # matmul free size per chunk
    assert CF <= 512
    M = B * HWl                 # BN normalization count (global batch)
    K9 = 9 * C
    HH = HWl // 2               # half-sample spatial size

    nc = bacc.Bacc("TRN2", target_bir_lowering=False, debug=False,
                   num_devices=n_cores)

    x_d = nc.dram_tensor("x", [npc, C, HWl], f32, kind="ExternalInput")
    w1_d = nc.dram_tensor("w1t", [C, K9], f32, kind="ExternalInput")
    w2_d = nc.dram_tensor("w2t", [C, K9], f32, kind="ExternalInput")
    par_d = nc.dram_tensor("params", [C, 4], f32, kind="ExternalInput")
    eye_d = nc.dram_tensor("eye8", [n_cores, n_cores], f32, kind="ExternalInput")
    eyeC_d = nc.dram_tensor("eyeC", [C, C], f32, kind="ExternalInput")
    out_d = nc.dram_tensor("out", [npc, C, HWl], f32, kind="ExternalOutput")

    groups = [list(range(n_cores))]

    with tile.TileContext(nc) as tc:
        with (
            tc.tile_pool(name="const", bufs=1) as constp,
            tc.tile_pool(name="xs", bufs=1) as xsp,
            tc.tile_pool(name="act", bufs=1) as actp,
            tc.tile_pool(name="z", bufs=1) as zp,
            tc.tile_pool(name="u", bufs=1) as up,
            tc.tile_pool(name="small", bufs=1) as smallp,
            tc.tile_pool(name="sq", bufs=3) as sqp,
            tc.tile_pool(name="psum", bufs=8, space="PSUM") as psump,
            tc.tile_pool(name="dram", bufs=1, space="DRAM") as dramp,
        ):
            def stile(tag, cols=1):
                return smallp.tile([C, cols], f32, tag=tag, name=tag)

            magic_t = stile("magic")
            nc.vector.memset(magic_t[:], MAGIC)
            nmagic_t = stile("nmagic")
            nc.vector.memset(nmagic_t[:], -MAGIC)
            eps_t = stile("eps")
            nc.vector.memset(eps_t[:], EPS)
            eye_sb = smallp.tile([n_cores, n_cores], f32, tag="eye8",
                                 name="eye8")
            nc.sync.dma_start(eye_sb[:], eye_d[:])
            eyeCf = constp.tile([C, C], f32, tag="eyeCf", name="eyeCf")
            nc.sync.dma_start(eyeCf[:], eyeC_d[:])
            eyeC = constp.tile([C, C], bf16, tag="eyeC", name="eyeC")
            nc.scalar.activation(out=eyeC[:], in_=eyeCf[:], func=AF.Copy)
            par_sb = stile("params", 4)
            nc.sync.dma_start(par_sb[:], par_d[:])
            gamma1, beta1 = par_sb[:, 0:1], par_sb[:, 1:2]
            gamma2, beta2 = par_sb[:, 2:3], par_sb[:, 3:4]

            # helpers for padded buffers ------------------------------------
            def pad_memset(t):
                # head guard + top pad row; bottom pad row + tail guard; the
                # single shared pad column of interior rows
                nc.vector.memset(t[:, 0:WP + 1], 0.0)
                nc.vector.memset(t[:, 1 + (H + 1) * WP:XKLEN], 0.0)
                side = t[:, 1:1 + (H + 2) * WP].rearrange(
                    "p (r w) -> p r w", w=WP)
                nc.vector.memset(side[:, 1:H + 1, W:W + 1], 0.0)

            def valid_view(t):
                # [C, H, W] view of the valid cells of a padded buffer
                return t[:, 1 + WP:1 + (H + 1) * WP].rearrange(
                    "p (r w) -> p r w", w=WP)[:, :, 0:W]

            # allocate + zero the padded activation buffers at t=0 (their
            # halo stays zero through both convs)
            xk = []
            for n in range(npc):
                xkt = actp.tile([C, XKLEN], bf16, tag=f"act{n}", name=f"act{n}")
                pad_memset(xkt)
                xk.append(xkt)

            # ---------------- x: load shard in halves, local absmax --------
            xs = []
            xmaxh = stile("xmaxh", 2 * npc)
            for n in range(npc):
                t = xsp.tile([C, HWl], f32, tag=f"xs{n}", name=f"xs{n}")
                for h in range(2):
                    sl = slice(h * HH, (h + 1) * HH)
                    nc.sync.dma_start(t[:, sl], x_d[n][:, sl])
                    nc.vector.tensor_reduce(out=xmaxh[:, 2 * n + h:2 * n + h + 1],
                                            in_=t[:, sl], axis=AX.X, op=OP.max,
                                            apply_absolute_value=True)
                xs.append(t)
            xmax = stile("xmax")
            nc.vector.tensor_reduce(out=xmax[:], in_=xmaxh[:], axis=AX.X,
                                    op=OP.max)

            # ---------------- weights: load + quantize to integer bf16 ----
            # (rint passes ride the otherwise-idle ACT engine)
            wk = []     # bf16 integer lhsT weights [C, 9*C]
            wmaxg = []  # replicated per-tensor absmax [C,1]
            for j, w_d in enumerate((w1_d, w2_d)):
                wsb = constp.tile([C, K9], f32, tag=f"wsb{j}", name=f"wsb{j}")
                nc.sync.dma_start(wsb[:], w_d[:])
                wm = stile(f"wmax{j}")
                nc.vector.tensor_reduce(out=wm[:], in_=wsb[:], axis=AX.X,
                                        op=OP.max, apply_absolute_value=True)
                wmr = stile(f"wmaxr{j}")
                nc.gpsimd.partition_all_reduce(wmr[:], wm[:], channels=C,
                                               reduce_op=bass_isa.ReduceOp.max)
                wrec = stile(f"wrec{j}")
                nc.vector.reciprocal(wrec[:], wmr[:])
                cw = stile(f"cw{j}")
                nc.vector.tensor_scalar_mul(cw[:], wrec[:], QMAX)
                wtmp = constp.tile([C, K9], f32, tag=f"wtmp{j}", name=f"wtmp{j}")
                nc.scalar.activation(out=wtmp[:], in_=wsb[:], func=AF.Identity,
                                     bias=magic_t[:], scale=cw[:])
                wq = constp.tile([C, K9], bf16, tag=f"wk{j}", name=f"wk{j}")
                nc.scalar.activation(out=wq[:], in_=wtmp[:], func=AF.Identity,
                                     bias=nmagic_t[:], scale=1.0)
                wk.append(wq)
                wmaxg.append(wmr)
            # AllGather local per-channel maxima -> [8, C]; transpose+reduce
            # +partition-replicate gives the global scalar s_x.
            ccx_i = dramp.tile([C, 1], f32, tag="ccx_i", name="ccx_i")
            ccx_o = dramp.tile([n_cores, C], f32, tag="ccx_o", name="ccx_o")
            nc.sync.dma_start(ccx_i[:], xmax[:])
            nc.gpsimd.collective_compute("AllGather", OP.bypass,
                                         replica_groups=groups,
                                         ins=[ccx_i[:].opt()],
                                         outs=[ccx_o[:].opt()])
            gathx = smallp.tile([n_cores, C], f32, tag="gathx", name="gathx")
            nc.sync.dma_start(gathx[:], ccx_o[:])
            tpx = psump.tile([C, n_cores], f32, tag="ps", name="tpx")
            nc.tensor.transpose(tpx[:], gathx[:], eye_sb[:])
            xmaxg = stile("xmaxg")
            nc.vector.tensor_reduce(out=xmaxg[:], in_=tpx[:], axis=AX.X,
                                    op=OP.max)
            sx = stile("sx")
            nc.gpsimd.partition_all_reduce(sx[:], xmaxg[:], channels=C,
                                           reduce_op=bass_isa.ReduceOp.max)
            sxrec = stile("sxrec")
            nc.vector.reciprocal(sxrec[:], sx[:])
            cx = stile("cx")
            nc.vector.tensor_scalar_mul(cx[:], sxrec[:], QMAX)

            # ---------------- quantize x -> integer bf16 padded -----------
            # pass 1 on ACT (x*cx + MAGIC rounds to f32), pass 2 on DVE
            # (-MAGIC, cast bf16 into the padded layout). Sample 0 goes in
            # chunk-sized pieces so conv1 can start after the first two.
            for n in range(npc):
                xkt = xk[n]
                u = up.tile([C, HWl], f32, tag="u0", name="u0")
                nsplit = NCH if n == 0 else 2
                HRq = H // nsplit
                for h in range(nsplit):
                    rsl = slice(h * HRq * W, (h + 1) * HRq * W)
                    nc.scalar.activation(out=u[:, rsl], in_=xs[n][:, rsl],
                                         func=AF.Identity, bias=magic_t[:],
                                         scale=cx[:])
                    nc.vector.tensor_scalar(
                        out=valid_view(xkt)[:, h * HRq:(h + 1) * HRq, :],
                        in0=u[:, rsl].rearrange("p (r w) -> p r w", w=W),
                        scalar1=MAGIC, scalar2=None, op0=OP.subtract)

            # conv1 scale constants, off the critical path
            al1 = stile("al1")
            nc.vector.tensor_tensor(al1[:], sx[:], wmaxg[0], OP.mult)
            nc.vector.tensor_scalar_mul(al1[:], al1[:], 1.0 / (QMAX * QMAX))
            alsq1 = stile("alsq1")
            nc.vector.tensor_tensor(alsq1[:], al1[:], al1[:], OP.mult)
            algam1 = stile("algam1")
            nc.vector.tensor_tensor(algam1[:], al1[:], gamma1, OP.mult)

            # ---------------- conv pass helper ----------------------------
            # per chunk: 9 accumulated matmuls (PE); ACT drains PSUM->z with
            # the per-channel sum accumulated; DVE does sumsq (+extrema for
            # conv1). Per-sample partial folds keep the end-of-conv stats
            # reduction tiny.
            def conv(src_tiles, wq, z_tag, nstats, zdt=f32):
                ops = [OP.add, OP.add, OP.max, OP.min][:nstats]
                ch_st = [stile(f"{z_tag}cst{j}", npc * NCH)
                         for j in range(nstats)]
                sm_st = stile(f"{z_tag}sst", nstats * npc)
                z_tiles = []
                for n in range(npc):
                    zt = zp.tile([C, HWl], zdt, tag=f"{z_tag}{n}",
                                 name=f"{z_tag}{n}")
                    zv_all = zt[:].rearrange("p (r w) -> p r w", w=W)
                    for g in range(NCH):
                        ps = psump.tile([C, CF], f32, tag="ps", name="ps")
                        for kh in range(3):
                            for kw_ in range(3):
                                k = kh * 3 + kw_
                                off = (g * RPC + kh) * WP + kw_
                                nc.tensor.matmul(
                                    ps[:],
                                    wq[:, k * C:(k + 1) * C],
                                    src_tiles[n][:, off:off + CF],
                                    start=(k == 0), stop=(k == 8))
                        pv = ps[:].rearrange("p (r w) -> p r w",
                                             w=WP)[:, :, 0:W]
                        zv = zv_all[:, g * RPC:(g + 1) * RPC, :]
                        ci = n * NCH + g
                        nc.scalar.activation(out=zv, in_=pv, func=AF.Copy,
                                             accum_out=ch_st[0][:, ci:ci + 1])
                        sq = sqp.tile([C, RPC, W], f32, tag="sq", name="sq")
                        nc.vector.scalar_tensor_tensor(
                            out=sq[:], in0=zv, scalar=1.0, in1=zv,
                            op0=OP.mult, op1=OP.mult,
                            accum_out=ch_st[1][:, ci:ci + 1])
                        if nstats > 2:
                            nc.vector.tensor_reduce(
                                out=ch_st[2][:, ci:ci + 1], in_=zv,
                                axis=AX.XY, op=OP.max)
                            nc.vector.tensor_reduce(
                                out=ch_st[3][:, ci:ci + 1], in_=zv,
                                axis=AX.XY, op=OP.min)
                    for j, op in enumerate(ops):
                        nc.vector.tensor_reduce(
                            out=sm_st[:, j * npc + n:j * npc + n + 1],
                            in_=ch_st[j][:, n * NCH:(n + 1) * NCH],
                            axis=AX.X, op=op)
                    z_tiles.append(zt)
                gin = stile(f"{z_tag}gin", nstats)
                for j, op in enumerate(ops):
                    nc.vector.tensor_reduce(
                        out=gin[:, j:j + 1],
                        in_=sm_st[:, j * npc:(j + 1) * npc], axis=AX.X, op=op)
                return z_tiles, gin

            # cross-core stat exchange: AllGather + transpose + reduce ------
            def cc_gather_stats(tag, gin, nstats):
                cc_i = dramp.tile([C, nstats], f32, tag=f"cc{tag}_i",
                                  name=f"cc{tag}_i")
                cc_o = dramp.tile([n_cores, C, nstats], f32, tag=f"cc{tag}_o",
                                  name=f"cc{tag}_o")
                nc.sync.dma_start(cc_i[:], gin[:])
                nc.gpsimd.collective_compute("AllGather", OP.bypass,
                                             replica_groups=groups,
                                             ins=[cc_i[:].opt()],
                                             outs=[cc_o[:].opt()])
                gath = smallp.tile([n_cores, C * nstats], f32,
                                   tag=f"gath{tag}", name=f"gath{tag}")
                nc.sync.dma_start(
                    gath[:], cc_o[:].rearrange("r c s -> r (c s)"))
                gv = gath[:].rearrange("r (c s) -> r s c", s=nstats)
                red = stile(f"red{tag}", nstats)
                for j, op in enumerate([OP.add, OP.add, OP.max,
                                        OP.min][:nstats]):
                    tp = psump.tile([C, n_cores], f32, tag="ps", name="tp")
                    nc.tensor.transpose(tp[:], gv[:, j:j + 1, :], eye_sb[:])
                    nc.vector.tensor_reduce(out=red[:, j:j + 1], in_=tp[:],
                                            axis=AX.X, op=op)
                return red

            # BN affine: A = al*gamma*rsqrt(var*al^2+eps), B = beta - mean*A
            # (all in integer-z units; al = s_in*s_w/127^2 dequant scale)
            def bn_affine(tag, red, alsq, algam, beta):
                mean_r = stile(f"mean_{tag}")
                nc.vector.tensor_scalar_mul(mean_r[:], red[:, 0:1], 1.0 / M)
                nmean = stile(f"nmean_{tag}")
                nc.vector.tensor_scalar_mul(nmean[:], red[:, 0:1], -1.0 / M)
                eq = stile(f"eq_{tag}")
                nc.vector.tensor_scalar_mul(eq[:], red[:, 1:2], 1.0 / M)
                var_r = stile(f"var_{tag}")
                nc.vector.scalar_tensor_tensor(out=var_r[:], in0=mean_r[:],
                                               scalar=nmean[:], in1=eq[:],
                                               op0=OP.mult, op1=OP.add)
                var_t = stile(f"vart_{tag}")
                nc.vector.tensor_tensor(var_t[:], var_r[:], alsq[:], OP.mult)
                sd = stile(f"sd_{tag}")
                nc.scalar.activation(out=sd[:], in_=var_t[:], func=AF.Sqrt,
                                     bias=eps_t[:], scale=1.0)
                rsd = stile(f"rsd_{tag}")
                nc.vector.reciprocal(rsd[:], sd[:])
                A = stile(f"A_{tag}")
                nc.vector.tensor_tensor(A[:], rsd[:], algam[:], OP.mult)
                Bc = stile(f"B_{tag}")
                nc.vector.scalar_tensor_tensor(out=Bc[:], in0=A[:],
                                               scalar=nmean[:], in1=beta,
                                               op0=OP.mult, op1=OP.add)
                return A, Bc

            # ---------------- conv1 + BN1 ---------------------------------
            z1, gin1 = conv(xk, wk[0], "z", 4)
            red1 = cc_gather_stats("1", gin1, 4)
            A1, B1 = bn_affine("1", red1, alsq1, algam1, beta1)

            # s_a1 = global max of relu(z*A1+B1) via channel extrema
            c1 = stile("cand1")
            nc.vector.scalar_tensor_tensor(out=c1[:], in0=red1[:, 2:3],
                                           scalar=A1[:], in1=B1[:],
                                           op0=OP.mult, op1=OP.add)
            c2 = stile("cand2")
            nc.vector.scalar_tensor_tensor(out=c2[:], in0=red1[:, 3:4],
                                           scalar=A1[:], in1=B1[:],
                                           op0=OP.mult, op1=OP.add)
            cand = stile("cand")
            nc.vector.tensor_tensor(cand[:], c1[:], c2[:], OP.max)
            nc.vector.tensor_scalar_max(cand[:], cand[:], 0.0)
            sa1 = stile("sa1")
            nc.gpsimd.partition_all_reduce(sa1[:], cand[:], channels=C,
                                           reduce_op=bass_isa.ReduceOp.max)
            sa1rec = stile("sa1rec")
            nc.vector.reciprocal(sa1rec[:], sa1[:])
            q1 = stile("q1")
            nc.vector.tensor_scalar_mul(q1[:], sa1rec[:], QMAX)
            A1q = stile("A1q")
            nc.vector.tensor_tensor(A1q[:], A1[:], q1[:], OP.mult)
            B1q = stile("B1q")
            nc.vector.tensor_tensor(B1q[:], B1[:], q1[:], OP.mult)

            # ---------------- apply BN1+ReLU+quantize -> a1k ---------------
            # ACT: relu(z*A1q + B1q); DVE: (+MAGIC, -MAGIC) rint, cast bf16.
            # a1k reuses the xk buffers; their pad halo is still zero, so no
            # re-memset is needed.
            a1k = []
            for n in range(npc):
                a1t = actp.tile([C, XKLEN], bf16, tag=f"act{n}", name=f"act{n}")
                u = up.tile([C, HWl], f32, tag="u0", name="u0")
                nsplit = NCH if n == 0 else 2
                HR = H // nsplit
                for h in range(nsplit):
                    rsl = slice(h * HR * W, (h + 1) * HR * W)
                    nc.scalar.activation(out=u[:, rsl], in_=z1[n][:, rsl],
                                         func=AF.Relu, bias=B1q[:],
                                         scale=A1q[:])
                    nc.vector.tensor_scalar(
                        out=valid_view(a1t)[:, h * HR:(h + 1) * HR, :],
                        in0=u[:, rsl].rearrange("p (r w) -> p r w", w=W),
                        scalar1=MAGIC, scalar2=MAGIC,
                        op0=OP.add, op1=OP.subtract)
                a1k.append(a1t)

            # conv2 scale constants, off the critical path
            al2 = stile("al2")
            nc.vector.tensor_tensor(al2[:], sa1[:], wmaxg[1], OP.mult)
            nc.vector.tensor_scalar_mul(al2[:], al2[:], 1.0 / (QMAX * QMAX))
            alsq2 = stile("alsq2")
            nc.vector.tensor_tensor(alsq2[:], al2[:], al2[:], OP.mult)
            algam2 = stile("algam2")
            nc.vector.tensor_tensor(algam2[:], al2[:], gamma2, OP.mult)

            # ---------------- conv2 + BN2 ---------------------------------
            z2, gin2 = conv(a1k, wk[1], "y", 2, zdt=bf16)

            # residual operand: cast x -> bf16 into the a1k buffers (freed
            # once conv2 has consumed them - emitted AFTER conv2 so the
            # write-after-read ordering holds); the DVE passes execute in
            # the BN2 turnaround window
            xb = []
            for n in range(npc):
                xt = actp.tile([C, XKLEN], bf16, tag=f"act{n}", name=f"act{n}")
                for h in range(2):
                    sl = slice(h * HH, (h + 1) * HH)
                    nc.vector.tensor_scalar(out=xt[:, sl], in0=xs[n][:, sl],
                                            scalar1=0.0, scalar2=None,
                                            op0=OP.add)
                xb.append(xt)
            red2 = cc_gather_stats("2", gin2, 2)
            A2, B2 = bn_affine("2", red2, alsq2, algam2, beta2)

            # ---------------- residual + relu + store ----------------------
            # PE (idle here): psum = diag(A2) @ z2_chunk + eye @ x_chunk;
            # drain relu(psum + B2) on ACT/DVE alternating; store per chunk.
            diagA2 = constp.tile([C, C], bf16, tag="diagA2", name="diagA2")
            nc.vector.tensor_scalar(out=diagA2[:], in0=eyeC[:],
                                    scalar1=A2[:], scalar2=None, op0=OP.mult)
            CW = 448  # 8-row chunks, no pad column needed here
            for n in range(npc):
                zb = z2[n][:]
                xv = xb[n][:, 0:HWl]
                for g in range(NCH):
                    sl = slice(g * CW, (g + 1) * CW)
                    ps = psump.tile([C, CW], f32, tag="ps", name="ps")
                    nc.tensor.matmul(ps[:], diagA2[:], zb[:, sl],
                                     start=True, stop=False)
                    nc.tensor.matmul(ps[:], eyeC[:], xv[:, sl],
                                     start=False, stop=True)
                    if g % 2 == 0:
                        nc.scalar.activation(out=xs[n][:, sl], in_=ps[:],
                                             func=AF.Relu, bias=B2[:],
                                             scale=1.0)
                    else:
                        nc.vector.tensor_scalar(out=xs[n][:, sl], in0=ps[:],
                                                scalar1=B2[:], scalar2=0.0,
                                                op0=OP.add, op1=OP.max)
                    nc.sync.dma_start(out_d[n][:, sl], xs[n][:, sl])

    nc.compile()
    return nc


def prepare_inputs(x, w1, gamma1, beta1, w2, gamma2, beta2,
                   n_cores=N_CORES):
    """Host-side sharding / layout marshaling (no math)."""
    x = np.ascontiguousarray(np.asarray(x, dtype=np.float32))
    B, C, H, W = x.shape
    w1t = np.ascontiguousarray(
        np.asarray(w1, np.float32).transpose(1, 2, 3, 0).reshape(C, 9 * C))
    w2t = np.ascontiguousarray(
        np.asarray(w2, np.float32).transpose(1, 2, 3, 0).reshape(C, 9 * C))
    params = np.ascontiguousarray(np.stack(
        [np.asarray(gamma1, np.float32), np.asarray(beta1, np.float32),
         np.asarray(gamma2, np.float32), np.asarray(beta2, np.float32)],
        axis=1))
    eye8 = np.eye(n_cores, dtype=np.float32)
    eyeC = np.eye(C, dtype=np.float32)
    shards = np.split(x.reshape(B, C, H * W), n_cores, axis=0)
    in_maps = [{"x": np.ascontiguousarray(s), "w1t": w1t, "w2t": w2t,
                "params": params, "eye8": eye8, "eyeC": eyeC} for s in shards]
    return in_maps


_module_cache = {}


def _get_module(shape):
    if shape not in _module_cache:
        B, C, H, W = shape
        nc = build_module(B=B, C=C, H=H, W=W)
        nc.m = get_hw_module(nc.m)
        _module_cache[shape] = nc
    return _module_cache[shape]


def run_on_hw(inputs, trace=False, **kwargs):
    x = np.asarray(inputs["x"])
    B, C, H, W = x.shape
    nc = _get_module((B, C, H, W))
    in_maps = prepare_inputs(**inputs)
    res = bass_utils.run_bass_kernel_spmd(
        nc, in_maps, core_ids=list(range(N_CORES)), trace=trace, **kwargs)
    out = np.concatenate([r["out"] for r in res.results], axis=0)
    return out.reshape(B, C, H, W).astype(np.float32), res


def kernel(**inputs):
    out, _ = run_on_hw(inputs)
    return out
